# revision 1
# baseline (speedup 1.0000x reference)
"""CrossAttention Trainium2 kernel (8-core SPMD, batch x seq sharding).

Reference math (per batch b):
  q = x @ Wq ; k = ctx @ Wk ; v = ctx @ Wv        (heads H=16, dim_head D=64)
  scores = (q @ k^T) * D**-0.5 ; attn = softmax(scores, kv axis)
  out = (attn @ v) @ Wo + bo

Sharding: 8 cores = 4 batches x 2 halves of the query sequence (N=4096).
Each core computes one batch, 2048 queries, all 16 heads. K/V projections are
recomputed per n-half (2x replication, cheap). No collectives.

On-device layout is transposed: all host tensors are pre-transposed so that
contraction dims land on SBUF partitions. Dtypes: bf16 operands for
q/k/v/scores/AV matmuls (fp32 PSUM accumulation), fp32r for the output
projection, fp32 softmax normalization.
"""

from dataclasses import dataclass

import numpy as np
import ml_dtypes

import concourse.bass as bass
import concourse.mybir as mybir
import concourse.tile as tile
from concourse import bacc

F32 = mybir.dt.float32
F32R = mybir.dt.float32r
BF16 = mybir.dt.bfloat16
AF = mybir.ActivationFunctionType


@dataclass(frozen=True)
class Cfg:
    NB: int = 4      # n-blocks per core
    NW: int = 512    # n width per block (moving-operand width)
    FT: int = 8      # x feature tiles of 128 (QUERY_DIM/128)
    CT: int = 6      # ctx feature tiles of 128 (CONTEXT_DIM/128)
    H: int = 16      # heads
    D: int = 64      # dim per head
    MT: int = 8      # kv tiles of 128 (M/128)
    JT: int = 8      # output feature tiles of 128

    @property
    def HP(self):  # head pairs == q/k dcol tiles of 128
        return self.H // 2

    @property
    def M(self):
        return self.MT * 128

    @property
    def MW(self):  # m chunk width for kT matmuls
        return min(self.NW, self.M)

    @property
    def MC(self):
        return self.M // self.MW


FULL = Cfg()


def build_kernel(cfg: Cfg = FULL, dbg: bool = False):
    c = cfg
    nc = bacc.Bacc("TRN2", target_bir_lowering=False, debug=False)

    # DRAM I/O (per-core shapes)
    xT = nc.dram_tensor("xT", [c.NB, 128, c.FT, c.NW], BF16, kind="ExternalInput")
    ctxT = nc.dram_tensor("ctxT", [128, c.CT, c.M], BF16, kind="ExternalInput")
    wq = nc.dram_tensor("wq", [c.HP, 128, c.FT, 128], BF16, kind="ExternalInput")
    wk = nc.dram_tensor("wk", [c.HP, 128, c.CT, 128], BF16, kind="ExternalInput")
    wv = nc.dram_tensor("wv", [2, 128, c.CT, (c.H // 2) * c.D], BF16, kind="ExternalInput")
    wo = nc.dram_tensor("wo", [c.JT, 128, c.HP, 128], BF16, kind="ExternalInput")
    bo_t = nc.dram_tensor("bo_t", [128, c.JT], F32, kind="ExternalInput")
    outT = nc.dram_tensor("outT", [c.NB, 128, c.JT, c.NW], F32, kind="ExternalOutput")
    if dbg:
        dbg_q = nc.dram_tensor("dbg_q", [128, c.HP, c.NW], BF16, kind="ExternalOutput")
        dbg_k = nc.dram_tensor("dbg_k", [128, c.HP, c.M], BF16, kind="ExternalOutput")
        dbg_e = nc.dram_tensor("dbg_e", [2, 128, c.MT, c.NW], BF16, kind="ExternalOutput")
        dbg_av = nc.dram_tensor("dbg_av", [128, 2, c.NW], F32, kind="ExternalOutput")
        dbg_rbc = nc.dram_tensor("dbg_rbc", [64, 2, c.NW], F32, kind="ExternalOutput")
        dbg_r = nc.dram_tensor("dbg_r", [2, 2, c.NW], F32, kind="ExternalOutput")
        dbg_attn = nc.dram_tensor("dbg_attn", [64, c.H, c.NW], BF16, kind="ExternalOutput")

    VW = (c.H // 2) * c.D  # width of one v-projection half

    with tile.TileContext(nc) as tc:
        with (
            tc.tile_pool(name="persist", bufs=1) as persist,
            tc.tile_pool(name="wstream", bufs=3) as wstream,
            tc.tile_pool(name="nbuf", bufs=2) as nbuf,
            tc.tile_pool(name="hbuf", bufs=2) as hbuf,
            tc.tile_pool(name="nrm", bufs=2) as nrm,
            tc.tile_pool(name="obuf", bufs=1) as obuf,
            tc.tile_pool(name="dbounce", bufs=4, space="DRAM") as dbounce,
            tc.tile_pool(name="ps_acc", bufs=2, space="PSUM") as ps_acc,
            tc.tile_pool(name="ps_sc", bufs=2, space="PSUM") as ps_sc,
            tc.tile_pool(name="ps_av", bufs=2, space="PSUM") as ps_av,
        ):
            # ---- persistent tiles ----
            ctx_sb = persist.tile([128, c.CT, c.M], BF16)
            kT_all = persist.tile([128, c.HP, c.M], BF16)
            v_aug = persist.tile([128, c.MT, c.H, c.D + 1], BF16)
            bo_sb = persist.tile([128, c.JT], F32)

            nc.sync.dma_start(out=ctx_sb, in_=ctxT[:, :, :])
            nc.sync.dma_start(out=bo_sb, in_=bo_t[:, :])
            nc.vector.memset(v_aug[:, :, :, c.D : c.D + 1], 1.0)

            # ---- kT: kT_all[dpair, m] = (ctx @ Wk).T ----
            for dc in range(c.HP):
                wk_g = wstream.tile([128, c.CT, 128], BF16, tag="wk")
                nc.sync.dma_start(out=wk_g, in_=wk[dc])
                for mc in range(c.MC):
                    ps = ps_acc.tile([128, c.MW], F32, tag="acc")
                    msl = bass.ts(mc, c.MW)
                    for ct in range(c.CT):
                        nc.tensor.matmul(
                            ps[:, :], wk_g[:, ct, :], ctx_sb[:, ct, msl],
                            start=(ct == 0), stop=(ct == c.CT - 1),
                        )
                    nc.vector.tensor_copy(out=kT_all[:, dc, msl], in_=ps[:, :])

            # ---- v: v_aug[m_tile, h, 0:D] = ctx @ Wv (strided into aug) ----
            for dh in range(2):
                wv_g = wstream.tile([128, c.CT, VW], BF16, tag="wv")
                nc.sync.dma_start(out=wv_g, in_=wv[dh])
                for mt in range(c.MT):
                    ps = ps_acc.tile([128, VW], F32, tag="acc")
                    for ct in range(c.CT):
                        nc.tensor.matmul(
                            ps[:, :], ctx_sb[:, ct, bass.ts(mt, 128)], wv_g[:, ct, :],
                            start=(ct == 0), stop=(ct == c.CT - 1),
                        )
                    nc.vector.tensor_copy(
                        out=v_aug[:, mt, bass.ts(dh, c.H // 2), 0 : c.D],
                        in_=ps[:, :].rearrange("p (h d) -> p h d", d=c.D),
                    )

            # ---- per n-block ----
            for nb in range(c.NB):
                x_sb = nbuf.tile([128, c.FT, c.NW], BF16, tag="x", bufs=1)
                nc.sync.dma_start(out=x_sb, in_=xT[nb])

                # qT_all[dpair, n] = (x @ Wq).T, Wq pre-scaled by D**-0.5
                qT_all = nbuf.tile([128, c.HP, c.NW], BF16, tag="qT", bufs=1)
                for dc in range(c.HP):
                    wq_g = wstream.tile([128, c.FT, 128], BF16, tag="wq")
                    nc.sync.dma_start(out=wq_g, in_=wq[dc])
                    ps = ps_acc.tile([128, c.NW], F32, tag="acc")
                    for ft in range(c.FT):
                        nc.tensor.matmul(
                            ps[:, :], wq_g[:, ft, :], x_sb[:, ft, :],
                            start=(ft == 0), stop=(ft == c.FT - 1),
                        )
                    nc.vector.tensor_copy(out=qT_all[:, dc, :], in_=ps[:, :])

                attn_n = nrm.tile([128, c.HP, c.NW], BF16, tag="attn", bufs=1)

                for hp in range(c.HP):
                    av = ps_av.tile([128, 2, c.NW], F32, tag="av")
                    exp_ev = hbuf.tile([128, c.MT, c.NW], BF16, tag="exp")
                    exp_od = hbuf.tile([128, c.MT, c.NW], BF16, tag="expo")
                    exp_p = [exp_ev, exp_od]
                    # scores^T [m, n] for both heads of the pair, interleaved so
                    # the K=64 matmuls co-run on disjoint PE row groups
                    for mt in range(c.MT):
                        for par in range(2):
                            prow = slice(par * 64, par * 64 + 64)
                            ps = ps_sc.tile([128, c.NW], F32, tag="sc")
                            nc.tensor.matmul(
                                ps[:, :],
                                kT_all[prow, hp, bass.ts(mt, 128)],
                                qT_all[prow, hp, :],
                                start=True, stop=True,
                            )
                            nc.scalar.activation(
                                out=exp_p[par][:, mt, :], in_=ps[:, :], func=AF.Exp,
                            )
                    if dbg and nb == 0 and hp == 0:
                        nc.sync.dma_start(out=dbg_e[0], in_=exp_p[0][:, :, :])
                        nc.sync.dma_start(out=dbg_e[1], in_=exp_p[1][:, :, :])
                    # AV + row sums (ones column): [D+1, NW] into psum pair
                    for par in range(2):
                        h = 2 * hp + par
                        for mt in range(c.MT):
                            nc.tensor.matmul(
                                av[0 : c.D + 1, par, :],
                                v_aug[:, mt, h, :],
                                exp_p[par][:, mt, :],
                                start=(mt == 0), stop=(mt == c.MT - 1),
                            )
                    # normalize both heads of the pair
                    r_sb = nrm.tile([128, 2, c.NW], F32, tag="r", bufs=3)
                    nc.scalar.activation(
                        out=r_sb[c.D : c.D + 1, :, :],
                        in_=av[c.D : c.D + 1, :, :].rearrange("p a n -> p (a n)"),
                        func=AF.Ln,
                    )
                    nc.scalar.activation(
                        out=r_sb[c.D : c.D + 1, :, :],
                        in_=r_sb[c.D : c.D + 1, :, :].rearrange("p a n -> p (a n)"),
                        func=AF.Exp, scale=-1.0,
                    )
                    r_dram = dbounce.tile([1, 2, c.NW], F32, tag="rd")
                    nc.sync.dma_start(out=r_dram, in_=r_sb[c.D : c.D + 1, :, :])
                    r_bc = nrm.tile([64, 2, c.NW], F32, tag="rbc", bufs=3)
                    nc.sync.dma_start(
                        out=r_bc, in_=r_dram[:, :, :].to_broadcast([64, 2, c.NW])
                    )
                    nc.vector.tensor_mul(
                        out=attn_n[0:64, hp, :],
                        in0=av[0:64, 0, :],
                        in1=r_bc[:, 0, :],
                    )
                    sh_tmp = nrm.tile([64, c.NW], BF16, tag="sh", bufs=2)
                    nc.vector.tensor_mul(
                        out=sh_tmp[:, :],
                        in0=av[0:64, 1, :],
                        in1=r_bc[:, 1, :],
                    )
                    nc.sync.dma_start(out=attn_n[64:128, hp, :], in_=sh_tmp[:, :])
                    if dbg and nb == 0 and hp == 0:
                        av_dump = nrm.tile([128, 2, c.NW], F32, tag="avd", bufs=1)
                        nc.vector.tensor_copy(out=av_dump[0:65, :, :], in_=av[0:65, :, :])
                        nc.sync.dma_start(out=dbg_av[0:65, :, :], in_=av_dump[0:65, :, :])
                        nc.sync.dma_start(out=dbg_rbc[:, :, :], in_=r_bc)
                        nc.sync.dma_start(out=dbg_r[0:1], in_=s_row[c.D : c.D + 1, :, :])
                        nc.sync.dma_start(out=dbg_r[1:2], in_=r_sb[c.D : c.D + 1, :, :])

                if dbg and nb == 0:
                    nc.sync.dma_start(out=dbg_q[:, :, :], in_=qT_all[:, :, :])
                    nc.sync.dma_start(out=dbg_k[:, :, :], in_=kT_all[:, :, :])
                # output projection (fp32r) + bias
                for j in range(c.JT):
                    wo_g = wstream.tile([128, c.HP, 128], BF16, tag="wo")
                    nc.sync.dma_start(out=wo_g, in_=wo[j])
                    ps = ps_acc.tile([128, c.NW], F32, tag="acc")
                    for hp2 in range(c.HP):
                        nc.tensor.matmul(
                            ps[:, :], wo_g[:, hp2, :],
                            attn_n[:, hp2, :],
                            start=(hp2 == 0), stop=(hp2 == c.HP - 1),
                        )
                    out_sb = obuf.tile([128, c.NW], F32, tag="out", bufs=2)
                    nc.vector.tensor_scalar_add(
                        out=out_sb[:, :], in0=ps[:, :], scalar1=bo_sb[:, j : j + 1]
                    )
                    nc.sync.dma_start(out=outT[nb][:, j, :], in_=out_sb)

    nc.compile()
    return nc


# ---------------- host side ----------------

def _prep_inputs(x, context, Wq, Wk, Wv, Wo, bo, cfg: Cfg = FULL, n_cores: int = 8):
    """Build the 8 per-core input maps (host-side transposes)."""
    c = cfg
    bf = ml_dtypes.bfloat16
    scale = np.float32(c.D) ** np.float32(-0.5)
    QD, CD, INNER, OD = c.FT * 128, c.CT * 128, c.H * c.D, c.JT * 128
    NCORE = c.NB * c.NW

    wq_t = np.ascontiguousarray(
        (Wq.astype(np.float32) * scale).reshape(c.FT, 128, c.HP, 128).transpose(2, 1, 0, 3)
    ).astype(bf)
    wk_t = np.ascontiguousarray(
        Wk.reshape(c.CT, 128, c.HP, 128).transpose(2, 1, 0, 3)
    ).astype(bf)
    wv_t = np.ascontiguousarray(
        Wv.reshape(c.CT, 128, 2, (c.H // 2) * c.D).transpose(2, 1, 0, 3)
    ).astype(bf)
    # rows hd of Wo grouped as [hp][par*64+d]: row index = (2*hp+par)*64+d
    wo_t = np.ascontiguousarray(
        Wo.reshape(c.HP, 2 * c.D, c.JT, 128).transpose(2, 1, 0, 3)
    ).astype(bf)
    bo_tt = np.ascontiguousarray(bo.reshape(c.JT, 128).T).astype(np.float32)

    B = x.shape[0]
    n_halves = n_cores // B
    in_maps = []
    for core in range(n_cores):
        b = core // n_halves
        n0 = (core % n_halves) * NCORE
        xs = x[b, n0 : n0 + NCORE, :]  # [NCORE, QD]
        xT_c = np.ascontiguousarray(
            xs.reshape(c.NB, c.NW, c.FT, 128).transpose(0, 3, 2, 1)
        ).astype(bf)
        ctxT_c = np.ascontiguousarray(
            context[b].T.reshape(c.CT, 128, c.M).transpose(1, 0, 2)
        ).astype(bf)
        in_maps.append({
            "xT": xT_c, "ctxT": ctxT_c, "wq": wq_t, "wk": wk_t,
            "wv": wv_t, "wo": wo_t, "bo_t": bo_tt,
        })
    return in_maps


def _gather_output(results, B, N, cfg: Cfg = FULL, n_cores: int = 8):
    c = cfg
    OD = c.JT * 128
    NCORE = c.NB * c.NW
    n_halves = n_cores // B
    out = np.empty((B, N, OD), dtype=np.float32)
    for core in range(n_cores):
        b = core // n_halves
        n0 = (core % n_halves) * NCORE
        oT = results[core]["outT"]  # [NB, 128, JT, NW]
        out[b, n0 : n0 + NCORE, :] = (
            oT.transpose(0, 3, 2, 1).reshape(NCORE, OD)
        )
    return out


_NC_CACHE = {}


def kernel(x, context, Wq, Wk, Wv, Wo, bo):
    from concourse.bass_utils import run_bass_kernel_spmd

    cfg = FULL
    if "nc" not in _NC_CACHE:
        _NC_CACHE["nc"] = build_kernel(cfg)
    nc = _NC_CACHE["nc"]

    x = np.asarray(x, dtype=np.float32)
    context = np.asarray(context, dtype=np.float32)
    in_maps = _prep_inputs(
        x, context,
        np.asarray(Wq, np.float32), np.asarray(Wk, np.float32),
        np.asarray(Wv, np.float32), np.asarray(Wo, np.float32),
        np.asarray(bo, np.float32), cfg,
    )
    res = run_bass_kernel_spmd(nc, in_maps, core_ids=list(range(8)))
    return _gather_output(res.results, x.shape[0], x.shape[1], cfg)



# revision 6
# speedup vs baseline: 1.8046x; 1.8046x over previous
"""CrossAttention Trainium2 kernel (8-core SPMD, batch x seq sharding).

Reference math (per batch b):
  q = x @ Wq ; k = ctx @ Wk ; v = ctx @ Wv        (heads H=16, dim_head D=64)
  scores = (q @ k^T) * D**-0.5 ; attn = softmax(scores, kv axis)
  out = (attn @ v) @ Wo + bo

Sharding: 8 cores = 4 batches x 2 halves of the query sequence (N=4096).
Each core computes one batch, 2048 queries, all 16 heads. No collectives.

V2 design (cost-model driven):
  - scores^T [m, n] tiles in PSUM (pairs of m-tiles share one 2-bank PSUM
    tile so each Exp activation covers free-size 1024, halving Act fixed
    overhead).
  - AV is computed in [n, d] orientation (attn^T tile as stationary, v as
    65-wide moving operand with a ones column): matmul cost is charged by
    moving width only, so this halves AV tensor time vs the [d, n]
    orientation, and the softmax row-sum lands in PSUM as a per-partition
    scalar (no DRAM broadcast bounce).
  - normalization: DVE reciprocal of the row-sum + tensor_scalar_mul.
  - attn^T for the output projection via PE transpose (identity matmul),
    PSUM->SBUF copies on DVE.
  - software pipelining: each hp iteration emits scores(hp), AV(hp-1),
    norm+transpose(hp-2), one Q tile of the next block and one output
    projection tile of the previous block, so PE and Act stay dense.
"""

from dataclasses import dataclass

import numpy as np
import ml_dtypes

import concourse.bass as bass
import concourse.mybir as mybir
import concourse.tile as tile
from concourse import bacc
from concourse.masks import make_identity

F32 = mybir.dt.float32
BF16 = mybir.dt.bfloat16
AF = mybir.ActivationFunctionType


@dataclass(frozen=True)
class Cfg:
    NB: int = 4      # n-blocks per core
    NW: int = 512    # n width per block (moving-operand width)
    FT: int = 8      # x feature tiles of 128 (QUERY_DIM/128)
    CT: int = 6      # ctx feature tiles of 128 (CONTEXT_DIM/128)
    H: int = 16      # heads
    D: int = 64      # dim per head
    MT: int = 8      # kv tiles of 128 (M/128)
    JT: int = 8      # output feature tiles of 128

    @property
    def HP(self):  # head pairs == q/k dcol tiles of 128
        return self.H // 2

    @property
    def M(self):
        return self.MT * 128

    @property
    def MW(self):  # m chunk width for kT matmuls
        return min(self.NW, self.M)

    @property
    def MC(self):
        return self.M // self.MW

    @property
    def NS(self):  # n sub-tiles of 128 per block
        return self.NW // 128


FULL = Cfg()


def build_kernel(cfg: Cfg = FULL):
    c = cfg
    nc = bacc.Bacc("TRN2", target_bir_lowering=False, debug=False)

    # DRAM I/O (per-core shapes)
    xT = nc.dram_tensor("xT", [c.NB, 128, c.FT, c.NW], BF16, kind="ExternalInput")
    ctxT = nc.dram_tensor("ctxT", [128, c.CT, c.M], BF16, kind="ExternalInput")
    wq = nc.dram_tensor("wq", [c.HP, 128, c.FT, 128], BF16, kind="ExternalInput")
    wk = nc.dram_tensor("wk", [c.HP, 128, c.CT, 128], BF16, kind="ExternalInput")
    wv = nc.dram_tensor("wv", [2, 128, c.CT, (c.H // 2) * c.D], BF16, kind="ExternalInput")
    wo = nc.dram_tensor("wo", [c.JT, 128, c.HP, 128], BF16, kind="ExternalInput")
    bo_t = nc.dram_tensor("bo_t", [128, c.JT], F32, kind="ExternalInput")
    outT = nc.dram_tensor("outT", [c.NB, 128, c.JT, c.NW], F32, kind="ExternalOutput")

    VW = (c.H // 2) * c.D  # width of one v-projection half

    with tile.TileContext(nc) as tc:
        with (
            tc.tile_pool(name="persist", bufs=1) as persist,
            tc.tile_pool(name="wstream", bufs=3) as wstream,
            tc.tile_pool(name="nbuf", bufs=2) as nbuf,
            tc.tile_pool(name="hbuf", bufs=2) as hbuf,
            tc.tile_pool(name="nrm", bufs=6) as nrm,
            tc.tile_pool(name="obuf", bufs=2) as obuf,
            tc.tile_pool(name="ps_acc", bufs=2, space="PSUM") as ps_acc,
            tc.tile_pool(name="ps_sc", bufs=2, space="PSUM") as ps_sc,
            tc.tile_pool(name="ps_av", bufs=2, space="PSUM") as ps_av,
        ):
            # ---- persistent tiles ----
            ctx_sb = persist.tile([128, c.CT, c.M], BF16)
            kT_all = persist.tile([128, c.HP, c.M], BF16)
            v_aug = persist.tile([128, c.MT, c.H, c.D + 1], BF16)
            wq_sb = persist.tile([128, c.HP, c.FT, 128], BF16)
            wo_sb = persist.tile([128, c.JT, c.HP, 128], BF16)
            bo_sb = persist.tile([128, c.JT], F32)
            ident = persist.tile([128, 128], BF16)

            nc.sync.dma_start(out=ctx_sb, in_=ctxT[:, :, :])
            nc.sync.dma_start(out=bo_sb, in_=bo_t[:, :])
            for dc in range(c.HP):
                nc.sync.dma_start(out=wq_sb[:, dc, :, :], in_=wq[dc])
            for j in range(c.JT):
                nc.sync.dma_start(out=wo_sb[:, j, :, :], in_=wo[j])
            nc.vector.memset(v_aug[:, :, :, c.D : c.D + 1], 1.0)
            make_identity(nc, ident)

            # ---- kT: kT_all[dpair, m] = (ctx @ Wk).T ----
            for dc in range(c.HP):
                wk_g = wstream.tile([128, c.CT, 128], BF16, tag="wk")
                nc.sync.dma_start(out=wk_g, in_=wk[dc])
                for mc in range(c.MC):
                    ps = ps_acc.tile([128, c.MW], F32, tag="acc")
                    msl = bass.ts(mc, c.MW)
                    for ct in range(c.CT):
                        nc.tensor.matmul(
                            ps[:, :], wk_g[:, ct, :], ctx_sb[:, ct, msl],
                            start=(ct == 0), stop=(ct == c.CT - 1),
                        )
                    nc.vector.tensor_copy(out=kT_all[:, dc, msl], in_=ps[:, :])

            # ---- V-projection emitters (interleaved into block 0) ----
            wv_tiles = {}

            def emit_v_dma(dh):
                wv_g = wstream.tile([128, c.CT, VW], BF16, tag="wv", name="wv_g")
                nc.sync.dma_start(out=wv_g, in_=wv[dh])
                wv_tiles[dh] = wv_g

            def emit_v_unit(dh, mt):
                # v_aug[m_tile, h-half, 0:D] = ctx @ Wv (strided into aug)
                wv_g = wv_tiles[dh]
                ps = ps_acc.tile([128, VW], F32, tag="acc")
                for ct in range(c.CT):
                    nc.tensor.matmul(
                        ps[:, :], ctx_sb[:, ct, bass.ts(mt, 128)], wv_g[:, ct, :],
                        start=(ct == 0), stop=(ct == c.CT - 1),
                    )
                nc.vector.tensor_copy(
                    out=v_aug[:, mt, bass.ts(dh, c.H // 2), 0 : c.D],
                    in_=ps[:, :].rearrange("p (h d) -> p h d", d=c.D),
                )

            # ---- per-block emitters ----
            x_tiles = {}
            q_tiles = {}
            attn_tiles = {}
            avn_tiles = {}
            av_tiles = {}
            exp_tiles = {}

            def emit_x_dma(nb):
                x_sb = nbuf.tile([128, c.FT, c.NW], BF16, tag="x", name="x_sb")
                nc.sync.dma_start(out=x_sb, in_=xT[nb])
                x_tiles[nb] = x_sb

            def emit_q_tile(nb, dc):
                # qT[dpair, n] = (x @ Wq).T, Wq pre-scaled by D**-0.5
                if dc == 0:
                    q_tiles[nb] = nbuf.tile([128, c.HP, c.NW], BF16, tag="qT", name="qT")
                ps = ps_acc.tile([128, c.NW], F32, tag="acc")
                for ft in range(c.FT):
                    nc.tensor.matmul(
                        ps[:, :], wq_sb[:, dc, ft, :], x_tiles[nb][:, ft, :],
                        start=(ft == 0), stop=(ft == c.FT - 1),
                    )
                nc.vector.tensor_copy(out=q_tiles[nb][:, dc, :], in_=ps[:, :])

            def emit_scores_pair(nb, hp, mtp, par):
                # scores^T [m, n] for head 2*hp+par, m-tiles 2*mtp..2*mtp+1
                prow = slice(par * 64, par * 64 + 64)
                sc_ps = ps_sc.tile([128, 2, c.NW], F32, tag="sc", name="sc_ps")
                for i in range(2):
                    mt = 2 * mtp + i
                    nc.tensor.matmul(
                        sc_ps[:, i, :],
                        kT_all[prow, hp, bass.ts(mt, 128)],
                        q_tiles[nb][prow, hp, :],
                        start=True, stop=True,
                    )
                nc.scalar.activation(
                    out=exp_tiles[(nb, hp)][par][:, 2 * mtp : 2 * mtp + 2, :],
                    in_=sc_ps[:, :, :], func=AF.Exp,
                )

            def emit_av(nb, hp, ns):
                # av[n, 0:65] = sum_m exp^T[m, n] * v_aug[m, h, 0:65]
                # One PSUM bank per (hp, ns): par-0 AV at [0:65], par-1 AV at
                # [128:193], transpose scratch at [256:320] (bf16-bitcast).
                av_t = ps_av.tile([128, 512], F32, tag="av", name="av_t")
                av_tiles[(hp % 2, ns)] = av_t
                exp_p = exp_tiles[(nb, hp)]
                nsl = bass.ts(ns, 128)
                for par in range(2):
                    h = 2 * hp + par
                    for mt in range(c.MT):
                        nc.tensor.matmul(
                            av_t[:, 128 * par : 128 * par + c.D + 1],
                            exp_p[par][:, mt, nsl],
                            v_aug[:, mt, h, :],
                            start=(mt == 0), stop=(mt == c.MT - 1),
                        )

            def emit_norm_trans(nb, hp, ns):
                av_t = av_tiles[(hp % 2, ns)]
                avn = avn_tiles[nb]
                recip = nrm.tile([128, 2], F32, tag="recip", name="recip")
                nc.vector.reciprocal(
                    out=recip[:, :], in_=av_t[:, c.D : c.D + 129 : 128]
                )
                for par in range(2):
                    nc.vector.tensor_scalar_mul(
                        avn[:, ns, hp, bass.ts(par, c.D)],
                        av_t[:, 128 * par : 128 * par + c.D],
                        recip[:, par : par + 1],
                    )
                tr = av_t[:, 256:320].bitcast(BF16)
                nc.tensor.matmul(
                    tr, avn[:, ns, hp, :], ident[:, :], is_transpose=True
                )
                nc.vector.tensor_copy(
                    out=attn_tiles[nb][:, hp, bass.ts(ns, 128)], in_=tr
                )

            def emit_outproj(nb, j):
                ps = ps_acc.tile([128, c.NW], F32, tag="acc")
                for hp2 in range(c.HP):
                    nc.tensor.matmul(
                        ps[:, :], wo_sb[:, j, hp2, :], attn_tiles[nb][:, hp2, :],
                        start=(hp2 == 0), stop=(hp2 == c.HP - 1),
                    )
                out_sb = obuf.tile([128, c.NW], F32, tag="out")
                nc.vector.tensor_scalar_add(
                    out=out_sb[:, :], in0=ps[:, :], scalar1=bo_sb[:, j : j + 1]
                )
                nc.sync.dma_start(out=outT[nb][:, j, :], in_=out_sb)

            # ---- prologue: x(0) + Q(0) ----
            emit_x_dma(0)
            emit_v_dma(0)
            emit_v_dma(1)
            for dc in range(c.HP):
                emit_q_tile(0, dc)

            # ---- software-pipelined block loop ----
            for nb in range(c.NB):
                if nb + 1 < c.NB:
                    emit_x_dma(nb + 1)
                avn_tiles[nb] = nbuf.tile([128, c.NS, c.HP, 128], BF16, tag="avn", name="avn")
                attn_tiles[nb] = nbuf.tile([128, c.HP, c.NW], BF16, tag="attn", name="attn")

                for hp in range(c.HP):
                    exp_tiles[(nb, hp)] = [
                        hbuf.tile([128, c.MT, c.NW], BF16, tag="ev", name="exp_ev"),
                        hbuf.tile([128, c.MT, c.NW], BF16, tag="od", name="exp_od"),
                    ]
                    for mtp in range(c.MT // 2):
                        for par in range(2):
                            emit_scores_pair(nb, hp, mtp, par)
                        if nb == 0 and hp < 2:
                            # block 0, hp 0/1 carry the V projection:
                            # all of dh0 during hp 0, dh1 during hp 1, so
                            # every unit is emitted before any AV reads it
                            emit_v_unit(hp, 2 * mtp)
                            emit_v_unit(hp, 2 * mtp + 1)
                        if hp >= 1:
                            emit_av(nb, hp - 1, mtp)
                            emit_norm_trans(nb, hp - 1, mtp)
                    if nb + 1 < c.NB:
                        emit_q_tile(nb + 1, hp)
                    if nb >= 1:
                        emit_outproj(nb - 1, hp)

                # block tail: AV + norm + transpose of the last head pair
                for ns in range(c.NS):
                    emit_av(nb, c.HP - 1, ns)
                    emit_norm_trans(nb, c.HP - 1, ns)

            # ---- epilogue: output projection of the last block ----
            for j in range(c.JT):
                emit_outproj(c.NB - 1, j)

    nc.compile()
    return nc


# ---------------- host side ----------------

def _prep_inputs(x, context, Wq, Wk, Wv, Wo, bo, cfg: Cfg = FULL, n_cores: int = 8):
    """Build the 8 per-core input maps (host-side transposes)."""
    c = cfg
    bf = ml_dtypes.bfloat16
    scale = np.float32(c.D) ** np.float32(-0.5)
    NCORE = c.NB * c.NW

    wq_t = np.ascontiguousarray(
        (Wq.astype(np.float32) * scale).reshape(c.FT, 128, c.HP, 128).transpose(2, 1, 0, 3)
    ).astype(bf)
    wk_t = np.ascontiguousarray(
        Wk.reshape(c.CT, 128, c.HP, 128).transpose(2, 1, 0, 3)
    ).astype(bf)
    wv_t = np.ascontiguousarray(
        Wv.reshape(c.CT, 128, 2, (c.H // 2) * c.D).transpose(2, 1, 0, 3)
    ).astype(bf)
    # rows hd of Wo grouped as [hp][par*64+d]: row index = (2*hp+par)*64+d
    wo_t = np.ascontiguousarray(
        Wo.reshape(c.HP, 2 * c.D, c.JT, 128).transpose(2, 1, 0, 3)
    ).astype(bf)
    bo_tt = np.ascontiguousarray(bo.reshape(c.JT, 128).T).astype(np.float32)

    B = x.shape[0]
    n_halves = n_cores // B
    in_maps = []
    for core in range(n_cores):
        b = core // n_halves
        n0 = (core % n_halves) * NCORE
        xs = x[b, n0 : n0 + NCORE, :]  # [NCORE, QD]
        xT_c = np.ascontiguousarray(
            xs.reshape(c.NB, c.NW, c.FT, 128).transpose(0, 3, 2, 1)
        ).astype(bf)
        ctxT_c = np.ascontiguousarray(
            context[b].T.reshape(c.CT, 128, c.M).transpose(1, 0, 2)
        ).astype(bf)
        in_maps.append({
            "xT": xT_c, "ctxT": ctxT_c, "wq": wq_t, "wk": wk_t,
            "wv": wv_t, "wo": wo_t, "bo_t": bo_tt,
        })
    return in_maps


def _gather_output(results, B, N, cfg: Cfg = FULL, n_cores: int = 8):
    c = cfg
    OD = c.JT * 128
    NCORE = c.NB * c.NW
    n_halves = n_cores // B
    out = np.empty((B, N, OD), dtype=np.float32)
    for core in range(n_cores):
        b = core // n_halves
        n0 = (core % n_halves) * NCORE
        oT = results[core]["outT"]  # [NB, 128, JT, NW]
        out[b, n0 : n0 + NCORE, :] = (
            oT.transpose(0, 3, 2, 1).reshape(NCORE, OD)
        )
    return out


_NC_CACHE = {}


def kernel(x, context, Wq, Wk, Wv, Wo, bo):
    from concourse.bass_utils import run_bass_kernel_spmd

    cfg = FULL
    if "nc" not in _NC_CACHE:
        _NC_CACHE["nc"] = build_kernel(cfg)
    nc = _NC_CACHE["nc"]

    x = np.asarray(x, dtype=np.float32)
    context = np.asarray(context, dtype=np.float32)
    in_maps = _prep_inputs(
        x, context,
        np.asarray(Wq, np.float32), np.asarray(Wk, np.float32),
        np.asarray(Wv, np.float32), np.asarray(Wo, np.float32),
        np.asarray(bo, np.float32), cfg,
    )
    res = run_bass_kernel_spmd(nc, in_maps, core_ids=list(range(8)))
    return _gather_output(res.results, x.shape[0], x.shape[1], cfg)


# revision 7
# speedup vs baseline: 1.8713x; 1.0370x over previous
"""CrossAttention Trainium2 kernel (8-core SPMD, batch x seq sharding).

Reference math (per batch b):
  q = x @ Wq ; k = ctx @ Wk ; v = ctx @ Wv        (heads H=16, dim_head D=64)
  scores = (q @ k^T) * D**-0.5 ; attn = softmax(scores, kv axis)
  out = (attn @ v) @ Wo + bo

Sharding: 8 cores = 4 batches x 2 halves of the query sequence (N=4096).
Each core computes one batch, 2048 queries, all 16 heads. No collectives.

V3 design (cost-model driven):
  - scores^T [m, n] tiles in PSUM; pairs of m-tiles share one 2-bank PSUM
    tile so each Exp activation covers free-size 1024 (halves Act fixed
    overhead).
  - AV in [n, d] orientation (exp^T tile stationary, v 65-wide moving with
    a ones column): matmul cost is charged by moving width only, so this
    halves AV tensor time, and the softmax row-sum lands in PSUM as a
    per-partition scalar (no broadcast needed).
  - normalization: DVE reciprocal + tensor_scalar_mul; attn^T via PE
    transpose into a scratch range of the same PSUM bank as the AV pair.
  - fully software-pipelined single stream: every hp iteration carries
    scores(hp), AV(hp-1), norm+transpose(hp-1), plus one "drag-along"
    chain (K^T / V / Q of a later block / output projection of an earlier
    block) so the tensor engine never starves while Act does the exps.
"""

from dataclasses import dataclass

import numpy as np
import ml_dtypes

import concourse.bass as bass
import concourse.mybir as mybir
import concourse.tile as tile
from concourse import bacc
from concourse.masks import make_identity

F32 = mybir.dt.float32
BF16 = mybir.dt.bfloat16
AF = mybir.ActivationFunctionType


@dataclass(frozen=True)
class Cfg:
    NB: int = 4      # n-blocks per core
    NW: int = 512    # n width per block (moving-operand width)
    FT: int = 8      # x feature tiles of 128 (QUERY_DIM/128)
    CT: int = 6      # ctx feature tiles of 128 (CONTEXT_DIM/128)
    H: int = 16      # heads
    D: int = 64      # dim per head
    MT: int = 8      # kv tiles of 128 (M/128)
    JT: int = 8      # output feature tiles of 128

    @property
    def HP(self):  # head pairs == q/k dcol tiles of 128
        return self.H // 2

    @property
    def M(self):
        return self.MT * 128

    @property
    def MW(self):  # m chunk width for kT matmuls
        return min(self.NW, self.M)

    @property
    def MC(self):
        return self.M // self.MW

    @property
    def NS(self):  # n sub-tiles of 128 per block
        return self.NW // 128


FULL = Cfg()


def build_kernel(cfg: Cfg = FULL):
    c = cfg
    nc = bacc.Bacc("TRN2", target_bir_lowering=False, debug=False)

    # DRAM I/O (per-core shapes)
    xT = nc.dram_tensor("xT", [c.NB, 128, c.FT, c.NW], BF16, kind="ExternalInput")
    ctxT = nc.dram_tensor("ctxT", [128, c.CT, c.M], BF16, kind="ExternalInput")
    wq = nc.dram_tensor("wq", [c.HP, 128, c.FT, 128], BF16, kind="ExternalInput")
    wk = nc.dram_tensor("wk", [c.HP, 128, c.CT, 128], BF16, kind="ExternalInput")
    wv = nc.dram_tensor("wv", [2, 128, c.CT, (c.H // 2) * c.D], BF16, kind="ExternalInput")
    wo = nc.dram_tensor("wo", [c.JT, 128, c.HP, 128], BF16, kind="ExternalInput")
    bo_t = nc.dram_tensor("bo_t", [128, c.JT], F32, kind="ExternalInput")
    outT = nc.dram_tensor("outT", [c.NB, 128, c.JT, c.NW], F32, kind="ExternalOutput")

    VW = (c.H // 2) * c.D  # width of one v-projection half

    with tile.TileContext(nc) as tc:
        with (
            tc.tile_pool(name="persist", bufs=1) as persist,
            tc.tile_pool(name="nbuf", bufs=2) as nbuf,
            tc.tile_pool(name="hbuf", bufs=2) as hbuf,
            tc.tile_pool(name="nrm", bufs=6) as nrm,
            tc.tile_pool(name="obuf", bufs=2) as obuf,
            tc.tile_pool(name="ps_acc", bufs=2, space="PSUM") as ps_acc,
            tc.tile_pool(name="ps_sc", bufs=2, space="PSUM") as ps_sc,
            tc.tile_pool(name="ps_av", bufs=2, space="PSUM") as ps_av,
        ):
            # ---- persistent tiles ----
            ctx_sb = persist.tile([128, c.CT, c.M], BF16)
            kT_all = persist.tile([128, c.HP, c.M], BF16)
            v_aug = persist.tile([128, c.MT, c.H, c.D + 1], BF16)
            wq_sb = persist.tile([128, c.HP, c.FT, 128], BF16)
            wk_sb = persist.tile([128, c.HP, c.CT, 128], BF16)
            wv_sb = persist.tile([128, 2, c.CT, VW], BF16)
            wo_sb = persist.tile([128, c.JT, c.HP, 128], BF16)
            bo_sb = persist.tile([128, c.JT], F32)
            ident = persist.tile([128, 128], BF16)

            x_tiles = {}
            q_tiles = {}
            attn_tiles = {}
            avn_tiles = {}
            av_tiles = {}
            exp_tiles = {}

            def emit_x_dma(nb):
                x_sb = nbuf.tile([128, c.FT, c.NW], BF16, tag="x", name="x_sb")
                nc.sync.dma_start(out=x_sb, in_=xT[nb])
                x_tiles[nb] = x_sb

            # DMA queue order = need order: ctx -> wk -> x0 -> wq0 -> wv ->
            # wq1..7 -> wo -> bo (startup-critical transfers first).
            nc.sync.dma_start(out=ctx_sb, in_=ctxT[:, :, :])
            for dc in range(c.HP):
                nc.sync.dma_start(out=wk_sb[:, dc, :, :], in_=wk[dc])
            emit_x_dma(0)
            nc.sync.dma_start(out=wq_sb[:, 0, :, :], in_=wq[0])
            for dh in range(2):
                nc.sync.dma_start(out=wv_sb[:, dh, :, :], in_=wv[dh])
            for dc in range(1, c.HP):
                nc.sync.dma_start(out=wq_sb[:, dc, :, :], in_=wq[dc])
            for j in range(c.JT):
                nc.sync.dma_start(out=wo_sb[:, j, :, :], in_=wo[j])
            nc.sync.dma_start(out=bo_sb, in_=bo_t[:, :])
            nc.vector.memset(v_aug[:, :, :, c.D : c.D + 1], 1.0)
            make_identity(nc, ident)

            # ---- drag-along chain emitters (all atomic: alloc..copy) ----

            def emit_kT_half(dc, mc):
                # kT_all[dpair, m-half] = (ctx @ Wk[:, dc-cols]).T
                ps = ps_acc.tile([128, c.MW], F32, tag="acc", name="ps")
                msl = bass.ts(mc, c.MW)
                for ct in range(c.CT):
                    nc.tensor.matmul(
                        ps[:, :], wk_sb[:, dc, ct, :], ctx_sb[:, ct, msl],
                        start=(ct == 0), stop=(ct == c.CT - 1),
                    )
                nc.vector.tensor_copy(out=kT_all[:, dc, msl], in_=ps[:, :])

            def emit_v_unit(dh, mt):
                # v_aug[m_tile, h-half, 0:D] = ctx @ Wv (strided into aug)
                ps = ps_acc.tile([128, VW], F32, tag="acc", name="ps")
                for ct in range(c.CT):
                    nc.tensor.matmul(
                        ps[:, :], ctx_sb[:, ct, bass.ts(mt, 128)], wv_sb[:, dh, ct, :],
                        start=(ct == 0), stop=(ct == c.CT - 1),
                    )
                nc.vector.tensor_copy(
                    out=v_aug[:, mt, bass.ts(dh, c.H // 2), 0 : c.D],
                    in_=ps[:, :].rearrange("p (h d) -> p h d", d=c.D),
                )

            def emit_q_tile(nb, dc):
                # qT[dpair, n] = (x @ Wq).T, Wq pre-scaled by D**-0.5
                if dc == 0:
                    q_tiles[nb] = nbuf.tile([128, c.HP, c.NW], BF16, tag="qT", name="qT")
                ps = ps_acc.tile([128, c.NW], F32, tag="acc", name="ps")
                for ft in range(c.FT):
                    nc.tensor.matmul(
                        ps[:, :], wq_sb[:, dc, ft, :], x_tiles[nb][:, ft, :],
                        start=(ft == 0), stop=(ft == c.FT - 1),
                    )
                nc.vector.tensor_copy(out=q_tiles[nb][:, dc, :], in_=ps[:, :])

            def emit_outproj(nb, j, split_tail=False):
                ps = ps_acc.tile([128, c.NW], F32, tag="acc", name="ps")
                for hp2 in range(c.HP):
                    nc.tensor.matmul(
                        ps[:, :], wo_sb[:, j, hp2, :], attn_tiles[nb][:, hp2, :],
                        start=(hp2 == 0), stop=(hp2 == c.HP - 1),
                    )
                out_sb = obuf.tile([128, c.NW], F32, tag="out", name="out_sb")
                if split_tail:
                    half = c.NW // 2
                    for s in range(2):
                        sl = bass.ts(s, half)
                        nc.vector.tensor_scalar_add(
                            out=out_sb[:, sl], in0=ps[:, sl],
                            scalar1=bo_sb[:, j : j + 1],
                        )
                        nc.sync.dma_start(out=outT[nb][:, j, sl], in_=out_sb[:, sl])
                else:
                    nc.vector.tensor_scalar_add(
                        out=out_sb[:, :], in0=ps[:, :], scalar1=bo_sb[:, j : j + 1]
                    )
                    nc.sync.dma_start(out=outT[nb][:, j, :], in_=out_sb)

            # ---- attention emitters ----

            def emit_scores_pair(nb, hp, mtp, par):
                # scores^T [m, n] for head 2*hp+par, m-tiles 2*mtp..2*mtp+1
                prow = slice(par * 64, par * 64 + 64)
                sc_ps = ps_sc.tile([128, 2, c.NW], F32, tag="sc", name="sc_ps")
                for i in range(2):
                    mt = 2 * mtp + i
                    nc.tensor.matmul(
                        sc_ps[:, i, :],
                        kT_all[prow, hp, bass.ts(mt, 128)],
                        q_tiles[nb][prow, hp, :],
                        start=True, stop=True,
                    )
                nc.scalar.activation(
                    out=exp_tiles[(nb, hp)][par][:, 2 * mtp : 2 * mtp + 2, :],
                    in_=sc_ps[:, :, :], func=AF.Exp,
                )

            def emit_av(nb, hp, ns):
                # av[n, 0:65] = sum_m exp^T[m, n] * v_aug[m, h, 0:65]
                # One PSUM bank per (hp, ns): par-0 AV at [0:65], par-1 AV at
                # [128:193], transpose scratch at [256:320] (bf16-bitcast).
                av_t = ps_av.tile([128, 512], F32, tag="av", name="av_t")
                av_tiles[(hp % 2, ns)] = av_t
                exp_p = exp_tiles[(nb, hp)]
                nsl = bass.ts(ns, 128)
                for par in range(2):
                    h = 2 * hp + par
                    for mt in range(c.MT):
                        nc.tensor.matmul(
                            av_t[:, 128 * par : 128 * par + c.D + 1],
                            exp_p[par][:, mt, nsl],
                            v_aug[:, mt, h, :],
                            start=(mt == 0), stop=(mt == c.MT - 1),
                        )

            def emit_norm_trans(nb, hp, ns):
                av_t = av_tiles[(hp % 2, ns)]
                avn = avn_tiles[nb]
                recip = nrm.tile([128, 2], F32, tag="recip", name="recip")
                nc.vector.reciprocal(
                    out=recip[:, :], in_=av_t[:, c.D : c.D + 129 : 128]
                )
                for par in range(2):
                    nc.vector.tensor_scalar_mul(
                        avn[:, ns, hp, bass.ts(par, c.D)],
                        av_t[:, 128 * par : 128 * par + c.D],
                        recip[:, par : par + 1],
                    )
                tr = av_t[:, 256:320].bitcast(BF16)
                nc.tensor.matmul(
                    tr, avn[:, ns, hp, :], ident[:, :], is_transpose=True
                )
                nc.vector.tensor_copy(
                    out=attn_tiles[nb][:, hp, bass.ts(ns, 128)], in_=tr
                )

            # ---- drag-along schedule -------------------------------------
            # One atomic chain per (nb, hp, mtp) slot, balancing PE work
            # against the 8.3us/hp of Act exp time:
            #   block 0: kT / Q0 / V / Q1 chains (preamble folded in)
            #   block 1: Q2 + outproj(0)
            #   block 2: Q3 + outproj(1) j=0..3
            #   block 3: outproj(1) j=4..7 + outproj(2)
            def drag(nb, hp, mtp):
                if nb == 0:
                    if hp < 2:
                        # V projection: dh0 during hp0, dh1 during hp1
                        emit_v_unit(hp, 2 * mtp)
                        emit_v_unit(hp, 2 * mtp + 1)
                    if hp <= 5:
                        dc = hp + 2
                        if mtp == 0:
                            emit_kT_half(dc, 0)
                        elif mtp == 1:
                            emit_kT_half(dc, 1)
                        elif mtp == 3:
                            emit_q_tile(0, dc)
                    else:
                        emit_q_tile(1, 4 * (hp - 6) + mtp)
                elif nb == 1:
                    if mtp == 1:
                        emit_q_tile(2, hp)
                    elif mtp == 3:
                        emit_outproj(0, hp)
                elif nb == 2:
                    if mtp == 1:
                        emit_q_tile(3, hp)
                    elif mtp == 3 and hp % 2 == 0:
                        emit_outproj(1, hp // 2)
                elif nb == 3:
                    # chains: outproj(1) j=4..7 then outproj(2) j=0..7
                    chains = [(1, 4 + i) for i in range(4)] + [
                        (2, i) for i in range(c.JT)
                    ]
                    if hp < 4:
                        idx = 2 * hp + (1 if mtp >= 2 else 0)
                        do_it = mtp in (1, 3)
                    else:
                        idx = 8 + (hp - 4)
                        do_it = mtp == 1
                    if do_it:
                        bb, j = chains[idx]
                        emit_outproj(bb, j)

            # ---- prologue (PE warms up on kT/Q0 while DMAs land) ----
            emit_kT_half(0, 0)
            emit_kT_half(0, 1)
            emit_kT_half(1, 0)
            emit_kT_half(1, 1)
            emit_q_tile(0, 0)
            emit_q_tile(0, 1)

            # ---- software-pipelined block loop ----
            for nb in range(c.NB):
                if nb + 1 < c.NB:
                    emit_x_dma(nb + 1)
                avn_tiles[nb] = nbuf.tile(
                    [128, c.NS, c.HP, 128], BF16, tag="avn", bufs=1, name="avn"
                )
                attn_tiles[nb] = nbuf.tile(
                    [128, c.HP, c.NW], BF16, tag="attn", bufs=3, name="attn"
                )

                for hp in range(c.HP):
                    exp_tiles[(nb, hp)] = [
                        hbuf.tile([128, c.MT, c.NW], BF16, tag="ev", name="exp_ev"),
                        hbuf.tile([128, c.MT, c.NW], BF16, tag="od", name="exp_od"),
                    ]
                    for mtp in range(c.MT // 2):
                        emit_scores_pair(nb, hp, mtp, 0)
                        if hp >= 1:
                            emit_av(nb, hp - 1, mtp)
                            emit_norm_trans(nb, hp - 1, mtp)
                        emit_scores_pair(nb, hp, mtp, 1)
                        drag(nb, hp, mtp)

                # block tail: AV + norm + transpose of the last head pair
                for ns in range(c.NS):
                    emit_av(nb, c.HP - 1, ns)
                    emit_norm_trans(nb, c.HP - 1, ns)

            # ---- epilogue: output projection of the last block ----
            for j in range(c.JT):
                emit_outproj(c.NB - 1, j, split_tail=(j >= c.JT - 2))

    nc.compile()
    return nc


# ---------------- host side ----------------

def _prep_inputs(x, context, Wq, Wk, Wv, Wo, bo, cfg: Cfg = FULL, n_cores: int = 8):
    """Build the 8 per-core input maps (host-side transposes)."""
    c = cfg
    bf = ml_dtypes.bfloat16
    scale = np.float32(c.D) ** np.float32(-0.5)
    NCORE = c.NB * c.NW

    wq_t = np.ascontiguousarray(
        (Wq.astype(np.float32) * scale).reshape(c.FT, 128, c.HP, 128).transpose(2, 1, 0, 3)
    ).astype(bf)
    wk_t = np.ascontiguousarray(
        Wk.reshape(c.CT, 128, c.HP, 128).transpose(2, 1, 0, 3)
    ).astype(bf)
    wv_t = np.ascontiguousarray(
        Wv.reshape(c.CT, 128, 2, (c.H // 2) * c.D).transpose(2, 1, 0, 3)
    ).astype(bf)
    # rows hd of Wo grouped as [hp][par*64+d]: row index = (2*hp+par)*64+d
    wo_t = np.ascontiguousarray(
        Wo.reshape(c.HP, 2 * c.D, c.JT, 128).transpose(2, 1, 0, 3)
    ).astype(bf)
    bo_tt = np.ascontiguousarray(bo.reshape(c.JT, 128).T).astype(np.float32)

    B = x.shape[0]
    n_halves = n_cores // B
    in_maps = []
    for core in range(n_cores):
        b = core // n_halves
        n0 = (core % n_halves) * NCORE
        xs = x[b, n0 : n0 + NCORE, :]  # [NCORE, QD]
        xT_c = np.ascontiguousarray(
            xs.reshape(c.NB, c.NW, c.FT, 128).transpose(0, 3, 2, 1)
        ).astype(bf)
        ctxT_c = np.ascontiguousarray(
            context[b].T.reshape(c.CT, 128, c.M).transpose(1, 0, 2)
        ).astype(bf)
        in_maps.append({
            "xT": xT_c, "ctxT": ctxT_c, "wq": wq_t, "wk": wk_t,
            "wv": wv_t, "wo": wo_t, "bo_t": bo_tt,
        })
    return in_maps


def _gather_output(results, B, N, cfg: Cfg = FULL, n_cores: int = 8):
    c = cfg
    OD = c.JT * 128
    NCORE = c.NB * c.NW
    n_halves = n_cores // B
    out = np.empty((B, N, OD), dtype=np.float32)
    for core in range(n_cores):
        b = core // n_halves
        n0 = (core % n_halves) * NCORE
        oT = results[core]["outT"]  # [NB, 128, JT, NW]
        out[b, n0 : n0 + NCORE, :] = (
            oT.transpose(0, 3, 2, 1).reshape(NCORE, OD)
        )
    return out


_NC_CACHE = {}


def kernel(x, context, Wq, Wk, Wv, Wo, bo):
    from concourse.bass_utils import run_bass_kernel_spmd

    cfg = FULL
    if "nc" not in _NC_CACHE:
        _NC_CACHE["nc"] = build_kernel(cfg)
    nc = _NC_CACHE["nc"]

    x = np.asarray(x, dtype=np.float32)
    context = np.asarray(context, dtype=np.float32)
    in_maps = _prep_inputs(
        x, context,
        np.asarray(Wq, np.float32), np.asarray(Wk, np.float32),
        np.asarray(Wv, np.float32), np.asarray(Wo, np.float32),
        np.asarray(bo, np.float32), cfg,
    )
    res = run_bass_kernel_spmd(nc, in_maps, core_ids=list(range(8)))
    return _gather_output(res.results, x.shape[0], x.shape[1], cfg)


# revision 10
# speedup vs baseline: 1.8882x; 1.0090x over previous
"""CrossAttention Trainium2 kernel (8-core SPMD, batch x seq sharding).

Reference math (per batch b):
  q = x @ Wq ; k = ctx @ Wk ; v = ctx @ Wv        (heads H=16, dim_head D=64)
  scores = (q @ k^T) * D**-0.5 ; attn = softmax(scores, kv axis)
  out = (attn @ v) @ Wo + bo

Sharding: 8 cores = 4 batches x 2 halves of the query sequence (N=4096).
Each core computes one batch, 2048 queries, all 16 heads. No collectives.

V3 design (cost-model driven):
  - scores^T [m, n] tiles in PSUM; pairs of m-tiles share one 2-bank PSUM
    tile so each Exp activation covers free-size 1024 (halves Act fixed
    overhead).
  - AV in [n, d] orientation (exp^T tile stationary, v 65-wide moving with
    a ones column): matmul cost is charged by moving width only, so this
    halves AV tensor time, and the softmax row-sum lands in PSUM as a
    per-partition scalar (no broadcast needed).
  - normalization: DVE reciprocal + tensor_scalar_mul; attn^T via PE
    transpose into a scratch range of the same PSUM bank as the AV pair.
  - fully software-pipelined single stream: every hp iteration carries
    scores(hp), AV(hp-1), norm+transpose(hp-1), plus one "drag-along"
    chain (K^T / V / Q of a later block / output projection of an earlier
    block) so the tensor engine never starves while Act does the exps.
"""

from dataclasses import dataclass

import numpy as np
import ml_dtypes

import concourse.bass as bass
import concourse.mybir as mybir
import concourse.tile as tile
from concourse import bacc
from concourse.masks import make_identity

F32 = mybir.dt.float32
BF16 = mybir.dt.bfloat16
AF = mybir.ActivationFunctionType


@dataclass(frozen=True)
class Cfg:
    NB: int = 4      # n-blocks per core
    NW: int = 512    # n width per block (moving-operand width)
    FT: int = 8      # x feature tiles of 128 (QUERY_DIM/128)
    CT: int = 6      # ctx feature tiles of 128 (CONTEXT_DIM/128)
    H: int = 16      # heads
    D: int = 64      # dim per head
    MT: int = 8      # kv tiles of 128 (M/128)
    JT: int = 8      # output feature tiles of 128

    @property
    def HP(self):  # head pairs == q/k dcol tiles of 128
        return self.H // 2

    @property
    def M(self):
        return self.MT * 128

    @property
    def MW(self):  # m chunk width for kT matmuls
        return min(self.NW, self.M)

    @property
    def MC(self):
        return self.M // self.MW

    @property
    def NS(self):  # n sub-tiles of 128 per block
        return self.NW // 128


FULL = Cfg()


def build_kernel(cfg: Cfg = FULL):
    c = cfg
    nc = bacc.Bacc("TRN2", target_bir_lowering=False, debug=False)

    # DRAM I/O (per-core shapes)
    xT = nc.dram_tensor("xT", [c.NB, 128, c.FT, c.NW], BF16, kind="ExternalInput")
    ctxT = nc.dram_tensor("ctxT", [128, c.CT, c.M], BF16, kind="ExternalInput")
    wq = nc.dram_tensor("wq", [c.HP, 128, c.FT, 128], BF16, kind="ExternalInput")
    wk = nc.dram_tensor("wk", [c.HP, 128, c.CT, 128], BF16, kind="ExternalInput")
    wv = nc.dram_tensor("wv", [2, 128, c.CT, (c.H // 2) * c.D], BF16, kind="ExternalInput")
    wo = nc.dram_tensor("wo", [c.JT, 128, c.HP, 128], BF16, kind="ExternalInput")
    bo_t = nc.dram_tensor("bo_t", [128, c.JT], F32, kind="ExternalInput")
    outT = nc.dram_tensor("outT", [c.NB, 128, c.JT, c.NW], F32, kind="ExternalOutput")

    VW = (c.H // 2) * c.D  # width of one v-projection half

    with tile.TileContext(nc) as tc:
        with (
            tc.tile_pool(name="persist", bufs=1) as persist,
            tc.tile_pool(name="nbuf", bufs=2) as nbuf,
            tc.tile_pool(name="hbuf", bufs=2) as hbuf,
            tc.tile_pool(name="nrm", bufs=6) as nrm,
            tc.tile_pool(name="obuf", bufs=2) as obuf,
            tc.tile_pool(name="ps_acc", bufs=2, space="PSUM") as ps_acc,
            tc.tile_pool(name="ps_sc", bufs=2, space="PSUM") as ps_sc,
            tc.tile_pool(name="ps_av", bufs=2, space="PSUM") as ps_av,
        ):
            # ---- persistent tiles ----
            ctx_sb = persist.tile([128, c.CT, c.M], BF16)
            kT_all = persist.tile([128, c.HP, c.M], BF16)
            v_aug = persist.tile([128, c.MT, c.H, c.D + 1], BF16)
            wq_sb = persist.tile([128, c.HP, c.FT, 128], BF16)
            wk_sb = persist.tile([128, c.HP, c.CT, 128], BF16)
            wv_sb = persist.tile([128, 2, c.CT, VW], BF16)
            wo_sb = persist.tile([128, c.JT, c.HP, 128], BF16)
            bo_sb = persist.tile([128, c.JT], F32)
            ident = persist.tile([128, 128], BF16)

            x_tiles = {}
            q_tiles = {}
            attn_tiles = {}
            avn_tiles = {}
            av_tiles = {}
            exp_tiles = {}

            def emit_x_dma(nb):
                x_sb = nbuf.tile([128, c.FT, c.NW], BF16, tag="x", name="x_sb")
                nc.sync.dma_start(out=x_sb, in_=xT[nb])
                x_tiles[nb] = x_sb

            # DMA queue order = need order: ctx (halves) -> wk -> x0 -> wq0 ->
            # wv -> wq1..7 -> wo -> bo (startup-critical transfers first).
            nc.sync.dma_start(out=ctx_sb[:, :, 0 : c.MW], in_=ctxT[:, :, 0 : c.MW])
            for dc in range(c.HP):
                nc.sync.dma_start(out=wk_sb[:, dc, :, :], in_=wk[dc])
            nc.sync.dma_start(out=ctx_sb[:, :, c.MW :], in_=ctxT[:, :, c.MW :])
            emit_x_dma(0)
            nc.sync.dma_start(out=wq_sb[:, 0, :, :], in_=wq[0])
            for dh in range(2):
                nc.sync.dma_start(out=wv_sb[:, dh, :, :], in_=wv[dh])
            for dc in range(1, c.HP):
                nc.sync.dma_start(out=wq_sb[:, dc, :, :], in_=wq[dc])
            for j in range(c.JT):
                nc.sync.dma_start(out=wo_sb[:, j, :, :], in_=wo[j])
            nc.sync.dma_start(out=bo_sb, in_=bo_t[:, :])
            nc.vector.memset(v_aug[:, :, :, c.D : c.D + 1], 1.0)
            make_identity(nc, ident)

            # ---- drag-along chain emitters (all atomic: alloc..copy) ----

            def emit_kT_half(dc, mc):
                # kT_all[dpair, m-half] = (ctx @ Wk[:, dc-cols]).T
                ps = ps_acc.tile([128, c.MW], F32, tag="acc", name="ps")
                msl = bass.ts(mc, c.MW)
                for ct in range(c.CT):
                    nc.tensor.matmul(
                        ps[:, :], wk_sb[:, dc, ct, :], ctx_sb[:, ct, msl],
                        start=(ct == 0), stop=(ct == c.CT - 1),
                    )
                nc.vector.tensor_copy(out=kT_all[:, dc, msl], in_=ps[:, :])

            def emit_v_unit(dh, mt):
                # v_aug[m_tile, h-half, 0:D] = ctx @ Wv (strided into aug)
                ps = ps_acc.tile([128, VW], F32, tag="acc", name="ps")
                for ct in range(c.CT):
                    nc.tensor.matmul(
                        ps[:, :], ctx_sb[:, ct, bass.ts(mt, 128)], wv_sb[:, dh, ct, :],
                        start=(ct == 0), stop=(ct == c.CT - 1),
                    )
                nc.vector.tensor_copy(
                    out=v_aug[:, mt, bass.ts(dh, c.H // 2), 0 : c.D],
                    in_=ps[:, :].rearrange("p (h d) -> p h d", d=c.D),
                )

            def emit_q_tile(nb, dc):
                # qT[dpair, n] = (x @ Wq).T, Wq pre-scaled by D**-0.5
                if dc == 0:
                    q_tiles[nb] = nbuf.tile([128, c.HP, c.NW], BF16, tag="qT", name="qT")
                ps = ps_acc.tile([128, c.NW], F32, tag="acc", name="ps")
                for ft in range(c.FT):
                    nc.tensor.matmul(
                        ps[:, :], wq_sb[:, dc, ft, :], x_tiles[nb][:, ft, :],
                        start=(ft == 0), stop=(ft == c.FT - 1),
                    )
                nc.vector.tensor_copy(out=q_tiles[nb][:, dc, :], in_=ps[:, :])

            def emit_outproj(nb, j, split_tail=False):
                ps = ps_acc.tile([128, c.NW], F32, tag="acc", name="ps")
                for hp2 in range(c.HP):
                    nc.tensor.matmul(
                        ps[:, :], wo_sb[:, j, hp2, :], attn_tiles[nb][:, hp2, :],
                        start=(hp2 == 0), stop=(hp2 == c.HP - 1),
                    )
                out_sb = obuf.tile([128, c.NW], F32, tag="out", name="out_sb")
                if split_tail:
                    # epilogue: halve the bias/store chunks and issue the
                    # stores on the (idle) Act HWDGE queue to shorten the
                    # final drain
                    half = c.NW // 2
                    for s in range(2):
                        sl = bass.ts(s, half)
                        nc.vector.tensor_scalar_add(
                            out=out_sb[:, sl], in0=ps[:, sl],
                            scalar1=bo_sb[:, j : j + 1],
                        )
                        nc.scalar.dma_start(out=outT[nb][:, j, sl], in_=out_sb[:, sl])
                else:
                    nc.vector.tensor_scalar_add(
                        out=out_sb[:, :], in0=ps[:, :], scalar1=bo_sb[:, j : j + 1]
                    )
                    nc.sync.dma_start(out=outT[nb][:, j, :], in_=out_sb)

            # ---- attention emitters ----

            def emit_scores_pair(nb, hp, mtp, par):
                # scores^T [m, n] for head 2*hp+par, m-tiles 2*mtp..2*mtp+1
                prow = slice(par * 64, par * 64 + 64)
                sc_ps = ps_sc.tile([128, 2, c.NW], F32, tag="sc", name="sc_ps")
                for i in range(2):
                    mt = 2 * mtp + i
                    nc.tensor.matmul(
                        sc_ps[:, i, :],
                        kT_all[prow, hp, bass.ts(mt, 128)],
                        q_tiles[nb][prow, hp, :],
                        start=True, stop=True,
                    )
                nc.scalar.activation(
                    out=exp_tiles[(nb, hp)][par][:, 2 * mtp : 2 * mtp + 2, :],
                    in_=sc_ps[:, :, :], func=AF.Exp,
                )

            def emit_av(nb, hp, ns):
                # av[n, 0:65] = sum_m exp^T[m, n] * v_aug[m, h, 0:65]
                # One PSUM bank per (hp, ns): par-0 AV at [0:65], par-1 AV at
                # [128:193], transpose scratch at [256:320] (bf16-bitcast).
                av_t = ps_av.tile([128, 512], F32, tag="av", name="av_t")
                av_tiles[(hp % 2, ns)] = av_t
                exp_p = exp_tiles[(nb, hp)]
                nsl = bass.ts(ns, 128)
                for par in range(2):
                    h = 2 * hp + par
                    for mt in range(c.MT):
                        nc.tensor.matmul(
                            av_t[:, 128 * par : 128 * par + c.D + 1],
                            exp_p[par][:, mt, nsl],
                            v_aug[:, mt, h, :],
                            start=(mt == 0), stop=(mt == c.MT - 1),
                        )

            def emit_norm_trans(nb, hp, ns):
                av_t = av_tiles[(hp % 2, ns)]
                avn = avn_tiles[nb]
                recip = nrm.tile([128, 2], F32, tag="recip", name="recip")
                nc.vector.reciprocal(
                    out=recip[:, :], in_=av_t[:, c.D : c.D + 129 : 128]
                )
                for par in range(2):
                    nc.vector.tensor_scalar_mul(
                        avn[:, ns, hp, bass.ts(par, c.D)],
                        av_t[:, 128 * par : 128 * par + c.D],
                        recip[:, par : par + 1],
                    )
                tr = av_t[:, 256:320].bitcast(BF16)
                nc.tensor.matmul(
                    tr, avn[:, ns, hp, :], ident[:, :], is_transpose=True
                )
                nc.vector.tensor_copy(
                    out=attn_tiles[nb][:, hp, bass.ts(ns, 128)], in_=tr
                )

            # ---- drag-along schedule -------------------------------------
            # One atomic chain per (nb, hp, mtp) slot, balancing PE work
            # against the 8.3us/hp of Act exp time:
            #   block 0: kT / Q0 / V / Q1 chains (preamble folded in)
            #   block 1: Q2 + outproj(0)
            #   block 2: Q3 + outproj(1) j=0..3
            #   block 3: outproj(1) j=4..7 + outproj(2)
            def drag(nb, hp, mtp):
                if nb == 0:
                    if hp < 2:
                        # V projection: dh0 during hp0, dh1 during hp1
                        emit_v_unit(hp, 2 * mtp)
                        emit_v_unit(hp, 2 * mtp + 1)
                    if hp <= 5:
                        dc = hp + 2
                        if mtp == 0:
                            emit_kT_half(dc, 0)
                        elif mtp == 1:
                            emit_kT_half(dc, 1)
                        elif mtp == 3:
                            emit_q_tile(0, dc)
                    else:
                        emit_q_tile(1, 4 * (hp - 6) + mtp)
                elif nb == 1:
                    if mtp == 1:
                        emit_q_tile(2, hp)
                    elif mtp == 3:
                        emit_outproj(0, hp)
                elif nb == 2:
                    if mtp == 1:
                        emit_q_tile(3, hp)
                    elif mtp == 3 and hp % 2 == 0:
                        emit_outproj(1, hp // 2)
                elif nb == 3:
                    # chains: outproj(1) j=4..7 then outproj(2) j=0..7
                    chains = [(1, 4 + i) for i in range(4)] + [
                        (2, i) for i in range(c.JT)
                    ]
                    if hp < 4:
                        idx = 2 * hp + (1 if mtp >= 2 else 0)
                        do_it = mtp in (1, 3)
                    else:
                        idx = 8 + (hp - 4)
                        do_it = mtp == 1
                    if do_it:
                        bb, j = chains[idx]
                        emit_outproj(bb, j)

            # ---- prologue (PE warms up on kT/Q0 while DMAs land) ----
            # mc=0 halves first: they only need the first ctx half
            emit_kT_half(0, 0)
            emit_kT_half(1, 0)
            emit_kT_half(0, 1)
            emit_kT_half(1, 1)
            emit_q_tile(0, 0)
            emit_q_tile(0, 1)

            # ---- software-pipelined block loop ----
            for nb in range(c.NB):
                if nb + 1 < c.NB:
                    emit_x_dma(nb + 1)
                avn_tiles[nb] = nbuf.tile(
                    [128, c.NS, c.HP, 128], BF16, tag="avn", bufs=1, name="avn"
                )
                attn_tiles[nb] = nbuf.tile(
                    [128, c.HP, c.NW], BF16, tag="attn", bufs=3, name="attn"
                )

                for hp in range(c.HP):
                    exp_tiles[(nb, hp)] = [
                        hbuf.tile([128, c.MT, c.NW], BF16, tag="ev", name="exp_ev"),
                        hbuf.tile([128, c.MT, c.NW], BF16, tag="od", name="exp_od"),
                    ]
                    for mtp in range(c.MT // 2):
                        emit_scores_pair(nb, hp, mtp, 0)
                        if hp >= 1:
                            emit_av(nb, hp - 1, mtp)
                            emit_norm_trans(nb, hp - 1, mtp)
                        emit_scores_pair(nb, hp, mtp, 1)
                        drag(nb, hp, mtp)

                # block tail: AV + norm + transpose of the last head pair
                for ns in range(c.NS):
                    emit_av(nb, c.HP - 1, ns)
                    emit_norm_trans(nb, c.HP - 1, ns)

            # ---- epilogue: output projection of the last block ----
            for j in range(c.JT):
                emit_outproj(c.NB - 1, j, split_tail=(j >= c.JT - 2))

    nc.compile()
    return nc


# ---------------- host side ----------------

def _prep_inputs(x, context, Wq, Wk, Wv, Wo, bo, cfg: Cfg = FULL, n_cores: int = 8):
    """Build the 8 per-core input maps (host-side transposes)."""
    c = cfg
    bf = ml_dtypes.bfloat16
    scale = np.float32(c.D) ** np.float32(-0.5)
    NCORE = c.NB * c.NW

    wq_t = np.ascontiguousarray(
        (Wq.astype(np.float32) * scale).reshape(c.FT, 128, c.HP, 128).transpose(2, 1, 0, 3)
    ).astype(bf)
    wk_t = np.ascontiguousarray(
        Wk.reshape(c.CT, 128, c.HP, 128).transpose(2, 1, 0, 3)
    ).astype(bf)
    wv_t = np.ascontiguousarray(
        Wv.reshape(c.CT, 128, 2, (c.H // 2) * c.D).transpose(2, 1, 0, 3)
    ).astype(bf)
    # rows hd of Wo grouped as [hp][par*64+d]: row index = (2*hp+par)*64+d
    wo_t = np.ascontiguousarray(
        Wo.reshape(c.HP, 2 * c.D, c.JT, 128).transpose(2, 1, 0, 3)
    ).astype(bf)
    bo_tt = np.ascontiguousarray(bo.reshape(c.JT, 128).T).astype(np.float32)

    B = x.shape[0]
    n_halves = n_cores // B
    in_maps = []
    for core in range(n_cores):
        b = core // n_halves
        n0 = (core % n_halves) * NCORE
        xs = x[b, n0 : n0 + NCORE, :]  # [NCORE, QD]
        xT_c = np.ascontiguousarray(
            xs.reshape(c.NB, c.NW, c.FT, 128).transpose(0, 3, 2, 1)
        ).astype(bf)
        ctxT_c = np.ascontiguousarray(
            context[b].T.reshape(c.CT, 128, c.M).transpose(1, 0, 2)
        ).astype(bf)
        in_maps.append({
            "xT": xT_c, "ctxT": ctxT_c, "wq": wq_t, "wk": wk_t,
            "wv": wv_t, "wo": wo_t, "bo_t": bo_tt,
        })
    return in_maps


def _gather_output(results, B, N, cfg: Cfg = FULL, n_cores: int = 8):
    c = cfg
    OD = c.JT * 128
    NCORE = c.NB * c.NW
    n_halves = n_cores // B
    out = np.empty((B, N, OD), dtype=np.float32)
    for core in range(n_cores):
        b = core // n_halves
        n0 = (core % n_halves) * NCORE
        oT = results[core]["outT"]  # [NB, 128, JT, NW]
        out[b, n0 : n0 + NCORE, :] = (
            oT.transpose(0, 3, 2, 1).reshape(NCORE, OD)
        )
    return out


_NC_CACHE = {}


def kernel(x, context, Wq, Wk, Wv, Wo, bo):
    from concourse.bass_utils import run_bass_kernel_spmd

    cfg = FULL
    if "nc" not in _NC_CACHE:
        _NC_CACHE["nc"] = build_kernel(cfg)
    nc = _NC_CACHE["nc"]

    x = np.asarray(x, dtype=np.float32)
    context = np.asarray(context, dtype=np.float32)
    in_maps = _prep_inputs(
        x, context,
        np.asarray(Wq, np.float32), np.asarray(Wk, np.float32),
        np.asarray(Wv, np.float32), np.asarray(Wo, np.float32),
        np.asarray(bo, np.float32), cfg,
    )
    res = run_bass_kernel_spmd(nc, in_maps, core_ids=list(range(8)))
    return _gather_output(res.results, x.shape[0], x.shape[1], cfg)
